# revision 7
# baseline (speedup 1.0000x reference)
"""Trainium2 Bass kernel for GQA attention (B=2, S=2048, D=2048, H=32, KVH=8).

Sharding: 8 cores = 2 batches x 4 head-groups. Each core handles one batch and
8 q-heads / 2 kv-heads: wq/wk/wv column-parallel, wo row-parallel; the partial
wo products are summed on the host.

All matmuls bf16 (fp8 was tried: its quantization noise lands at ~1.8e-2 final
error vs the 2e-2 gate - no margin). The speed comes from keeping the PE
busy continuously (its clock ramps 1.2->2.4GHz only after ~3us without a gap):

  - Host packs every input as [128 x contiguous] blocks so each DMA is ~128
    big descriptors; chunk-0 data is queued first so compute starts early.
  - q/k/v projections are fused into one rhs (wqkv, N=768 per d-tile).
  - The causal diagonal is handled by accumulating a constant -10000
    triangular tile into the scores PSUM via a K=128 identity matmul, so
    exp -> pv has no cross-engine mask hop.
  - The attention inner loop is software-pipelined (scores(j) emitted before
    pv(j-1)) and one unit of the PREVIOUS chunk's output projection is
    interleaved after each j so the PE never drains while ACT runs exp.
  - softmax denominators: PV lhsT carries a ones column, so Z accumulates in
    PSUM row 64; recip on ACT (bf16), broadcast across partitions with a
    K=1 bf16 matmul.
"""

import os
import sys

for _p in ("/opt/trn_rl_repo", "/root/.axon_site/_ro/trn_rl_repo"):
    if os.path.isdir(_p) and _p not in sys.path:
        sys.path.append(_p)

import numpy as np
import ml_dtypes

import concourse.bass as bass
import concourse.mybir as mybir
import concourse.tile as tile
from concourse import bacc, bass_utils
from concourse.masks import make_identity

F32 = mybir.dt.float32
BF16 = mybir.dt.bfloat16
AFT = mybir.ActivationFunctionType

P = 128
D = 2048
HD = 64
NJ = HD // 2          # 32 rope freqs
OQ = 512              # q-head dims per core (8 heads * 64)
OKV = 128             # kv-head dims per core (2 heads * 64)
NPAIR = 4             # head pairs per core
DT = D // P           # 16 d-tiles
OW = OQ + 256         # fused qkv projection width (512 q + 128 k + 128 v)

HEAD_PERM = [0, 4, 1, 5, 2, 6, 3, 7]


def _act_recip(nc, out_ap, in_ap):
    eng = nc.scalar
    ins = [eng.lower_ap(in_ap),
           mybir.ImmediateValue(dtype=mybir.dt.float32, value=0.0),
           mybir.ImmediateValue(dtype=mybir.dt.float32, value=1.0),
           mybir.ImmediateValue(dtype=mybir.dt.float32, value=0.0)]
    return eng.add_instruction(mybir.InstActivation(
        name=nc.get_next_instruction_name(), func=AFT.Reciprocal,
        ins=ins, outs=[eng.lower_ap(out_ap)]))


def _emit_rope(nc, out_sb, in_ap, cos_ap, sin_ap, nh, tmp_pool):
    """RoPE: out[.., 2j] = x0*c - x1*s ; out[.., 2j+1] = x0*s + x1*c.
    in_ap: [128, nh*64] f32 (psum); out_sb: [128, nh*64] bf16;
    cos_ap/sin_ap: [128, 32] (per s-tile)."""
    w = nh * NJ
    x = in_ap.rearrange("p (h j t) -> p h j t", h=nh, j=NJ, t=2)
    o = out_sb.rearrange("p (h j t) -> p h j t", h=nh, j=NJ, t=2)
    x0, x1 = x[:, :, :, 0], x[:, :, :, 1]
    o0, o1 = o[:, :, :, 0], o[:, :, :, 1]
    c = cos_ap.unsqueeze(1).broadcast_to([P, nh, NJ])
    s = sin_ap.unsqueeze(1).broadcast_to([P, nh, NJ])
    ta = tmp_pool.tile([P, w], F32, tag="rope_ta")
    tb = tmp_pool.tile([P, w], F32, tag="rope_tb")
    ta3 = ta.rearrange("p (h j) -> p h j", h=nh, j=NJ)
    tb3 = tb.rearrange("p (h j) -> p h j", h=nh, j=NJ)
    nc.vector.tensor_mul(ta3, x0, c)
    nc.vector.tensor_mul(tb3, x1, s)
    nc.vector.tensor_sub(o0, ta3, tb3)
    nc.vector.tensor_mul(ta3, x0, s)
    nc.vector.tensor_mul(tb3, x1, c)
    nc.vector.tensor_add(o1, ta3, tb3)


def emit_kernel(nc, tc, ctx, S):
    NSC = S // 512        # s-chunks
    NST = S // P          # s-tiles (global)
    CW = DT * 512         # free width of one chunk of xT in sbuf

    xTv_d = nc.dram_tensor("xTv", [NSC * P, CW], BF16, kind="ExternalInput").ap()
    wqkv_d = nc.dram_tensor("wqkv", [P, DT * OW], BF16, kind="ExternalInput").ap()
    wo_d = nc.dram_tensor("wo", [P, NPAIR * D], BF16, kind="ExternalInput").ap()
    cos_d = nc.dram_tensor("cost", [P, NST * NJ], F32, kind="ExternalInput").ap()
    sin_d = nc.dram_tensor("sint", [P, NST * NJ], F32, kind="ExternalInput").ap()
    out_d = nc.dram_tensor("out", [S, D], F32, kind="ExternalOutput").ap()

    ctx.enter_context(nc.allow_low_precision(reason="bf16 tiles feed matmuls"))
    const = ctx.enter_context(tc.tile_pool(name="const", bufs=1))
    work = ctx.enter_context(tc.tile_pool(name="work", bufs=2))
    epool = ctx.enter_context(tc.tile_pool(name="epool", bufs=6))
    qTp = ctx.enter_context(tc.tile_pool(name="qTp", bufs=2))
    atp = ctx.enter_context(tc.tile_pool(name="atp", bufs=2))
    psA = ctx.enter_context(tc.tile_pool(name="psA", bufs=2, space="PSUM"))
    psB = ctx.enter_context(tc.tile_pool(name="psB", bufs=4, space="PSUM"))

    idn = const.tile([P, P], BF16)
    make_identity(nc, idn)
    triU = const.tile([P, P], BF16)         # -10000 where k > q, else 0
    nc.gpsimd.memset(triU[:], -10000.0)
    nc.gpsimd.affine_select(out=triU[:], in_=triU[:],
                            compare_op=mybir.AluOpType.is_ge, fill=0.0,
                            base=-1, channel_multiplier=1, pattern=[[-1, P]])
    ones_f = const.tile([P, 1], F32)
    nc.any.memset(ones_f[:], 1.0)
    ones64 = const.tile([65, HD], BF16)
    nc.vector.tensor_copy(ones64[:], ones_f[0:65, 0:1].broadcast_to([65, HD]))

    xTv = const.tile([P, NSC * CW], BF16)   # [p, c*CW + dt*512 + sl]
    wqkv = const.tile([P, DT * OW], BF16)   # [p, dt*768 + (q512|k128|v128)]
    wo = const.tile([P, NPAIR * D], BF16)   # [p, pp*2048 + d]
    kT = const.tile([P, S], BF16)           # [o_kv, s]
    v2 = const.tile([P, NST * 130], BF16)   # [s_loc, g*130 + a*65 + (hd|one)]
    cosr = const.tile([P, NST * NJ], F32)
    sinr = const.tile([P, NST * NJ], F32)

    # priority-ordered bulk loads (each side is [128, contiguous])
    nc.sync.dma_start(wqkv[:], wqkv_d)
    nc.sync.dma_start(xTv[:, 0:CW], xTv_d[0:P, :])
    nc.sync.dma_start(cosr[:], cos_d)
    nc.sync.dma_start(sinr[:], sin_d)
    for c in range(1, NSC):
        nc.sync.dma_start(xTv[:, c * CW:(c + 1) * CW], xTv_d[c * P:(c + 1) * P, :])
    nc.sync.dma_start(wo[:], wo_d)

    # ones columns of v2 (positions i*65 + 64)
    v2ones = v2[:].rearrange("p (i c) -> p i c", i=2 * NST, c=65)[:, :, 64]
    nc.vector.tensor_copy(v2ones, ones_f[:, 0:1].broadcast_to([P, 2 * NST]))

    wqkvv = wqkv[:].rearrange("p (dt o) -> p dt o", dt=DT, o=OW)

    # ---- fused q/k/v projection for one s-chunk ----
    def emit_proj(c, qT):
        xvc = xTv[:, c * CW:(c + 1) * CW].rearrange("p (dt s) -> p dt s", dt=DT, s=512)
        pjs, tps = [], []
        for st in range(4):
            sl = slice(st * P, (st + 1) * P)
            pj = psA.tile([P, 1024], F32, tag="sc", bufs=2, name=f"pj_{c}_{st}")
            for dt in range(DT):
                nc.tensor.matmul(pj[:, 0:512], xvc[:, dt, sl], wqkvv[:, dt, 0:512],
                                 start=(dt == 0), stop=(dt == DT - 1),
                                 skip_group_check=True)
            for dt in range(DT):
                nc.tensor.matmul(pj[:, 512:OW], xvc[:, dt, sl], wqkvv[:, dt, 512:OW],
                                 start=(dt == 0), stop=(dt == DT - 1),
                                 skip_group_check=True)
            pjs.append(pj)
            if st > 0:
                emit_rope_tp(c, st - 1, pjs[st - 1], tps)
        emit_rope_tp(c, 3, pjs[3], tps)
        for st in range(4):
            emit_tp_copy(c, st, qT, tps[st])

    def emit_rope_tp(c, st, pj, tps):
        g = c * 4 + st
        cos_ap = cosr[:, g * NJ:(g + 1) * NJ]
        sin_ap = sinr[:, g * NJ:(g + 1) * NJ]
        qr = work.tile([P, OQ], BF16, tag="qr")
        _emit_rope(nc, qr[:], pj[:, 0:512], cos_ap, sin_ap, 8, work)
        kr = work.tile([P, OKV], BF16, tag="kr")
        _emit_rope(nc, kr[:], pj[:, 512:640], cos_ap, sin_ap, 2, work)
        v_src = pj[:, 640:768].rearrange("p (a x) -> p a x", a=2, x=HD)
        v_dst = v2[:, g * 130:(g + 1) * 130].rearrange("p (a x) -> p a x",
                                                       a=2, x=65)[:, :, 0:HD]
        nc.vector.tensor_copy(v_dst, v_src)
        # transposes into a borrowed psB slot (bf16, 1 cycle/row)
        tp = psB.tile([P, 640], BF16, tag="pv", bufs=4, name=f"tp_{c}_{st}")
        for p in range(NPAIR):
            nc.tensor.transpose(tp[:, p * P:(p + 1) * P], qr[:, p * P:(p + 1) * P], idn[:])
        nc.tensor.transpose(tp[:, 512:640], kr[:], idn[:])
        tps.append(tp)

    def emit_tp_copy(c, st, qT, tp):
        g = c * 4 + st
        for p in range(NPAIR):
            nc.vector.tensor_copy(qT[:, p * 512 + st * P: p * 512 + (st + 1) * P],
                                  tp[:, p * P:(p + 1) * P])
        nc.vector.tensor_copy(kT[:, g * P:(g + 1) * P], tp[:, 512:640])

    def emit_pv(pvt, prev, NJT):
        j, vs, e2s = prev
        for pp in range(2):
            e2 = e2s[pp]
            nc.tensor.matmul(pvt[(pp, 0)][:, vs:512],
                             v2[:, j * 130: j * 130 + 65],
                             e2[:, vs:512],
                             start=(j == 0), stop=(j == NJT - 1), skip_group_check=True)
            nc.tensor.matmul(pvt[(pp, 1)][:, vs:512],
                             v2[:, j * 130 + 65: (j + 1) * 130],
                             e2[:, 512 + vs:1024],
                             start=(j == 0), stop=(j == NJT - 1), skip_group_check=True)

    def emit_final_unit(fc, attnT_f, unit):
        st, dc = divmod(unit, 4)
        rp = psA.tile([P, 1024], F32, tag="sc", bufs=2, name=f"rp_{fc}_{unit}")
        rpv = rp[:, 0:512]
        for p in range(NPAIR):
            nc.tensor.matmul(rpv, attnT_f[:, p * 512 + st * P: p * 512 + (st + 1) * P],
                             wo[:, p * D + dc * 512: p * D + (dc + 1) * 512],
                             start=(p == 0), stop=(p == NPAIR - 1),
                             skip_group_check=True)
        rs = work.tile([P, 512], F32, tag="rs")
        nc.vector.tensor_copy(rs[:], rpv)
        nc.sync.dma_start(out_d[(fc * 4 + st) * P:(fc * 4 + st + 1) * P,
                                dc * 512:(dc + 1) * 512], rs[:])

    # ---- main loop over s-chunks ----
    qT_cur = qTp.tile([P, NPAIR * 512], BF16, tag="qT", name="qT_0")
    emit_proj(0, qT_cur)
    attnT_prev = None
    for c in range(NSC):
        qT = qT_cur
        attnT = atp.tile([P, NPAIR * 512], BF16, tag="attnT")
        NJT = 4 * (c + 1)
        fin_unit = 0 if attnT_prev is not None else 16
        for pg in range(2):          # two pair-groups, 2 head-pairs each
            pvt = {}
            for pp in range(2):
                for half in range(2):
                    pvt[(pp, half)] = psB.tile([65, 512], F32, tag="pv", bufs=4,
                                               name=f"pv_{c}_{pg}_{pp}_{half}")
            prev = None              # (j, vs, {pp: e2})
            for j in range(NJT):
                vs = max(0, (j - 4 * c) * P)
                e2s = {}
                for pp in range(2):
                    p = pg * 2 + pp
                    sc2 = psA.tile([P, 1024], F32, tag="sc", bufs=2)
                    nc.tensor.matmul(sc2[:, vs:512], kT[0:HD, j * P:(j + 1) * P],
                                     qT[0:HD, p * 512 + vs:(p + 1) * 512])
                    nc.tensor.matmul(sc2[:, 512 + vs:1024], kT[HD:P, j * P:(j + 1) * P],
                                     qT[HD:P, p * 512 + vs:(p + 1) * 512])
                    if j >= 4 * c:   # diagonal: accumulate -10000 upper-tri
                        for half in range(2):
                            nc.tensor.matmul(sc2[:, half * 512 + vs: half * 512 + vs + P],
                                             idn[:], triU[:],
                                             start=False, stop=True,
                                             skip_group_check=True)
                    e2 = epool.tile([P, 1024], BF16, tag="e", bufs=6)
                    if vs:
                        sc_v = sc2[:].rearrange("p (h q) -> p h q", h=2, q=512)[:, :, vs:512]
                        e_v = e2[:].rearrange("p (h q) -> p h q", h=2, q=512)[:, :, vs:512]
                        nc.scalar.activation(e_v, sc_v, AFT.Exp, scale=1.0 / 8.0)
                    else:
                        nc.scalar.activation(e2[:], sc2[:], AFT.Exp, scale=1.0 / 8.0)
                    e2s[pp] = e2
                if prev is not None:
                    emit_pv(pvt, prev, NJT)
                prev = (j, vs, e2s)
                if fin_unit < 16:
                    emit_final_unit(c - 1, attnT_prev, fin_unit)
                    fin_unit += 1
            emit_pv(pvt, prev, NJT)
            # normalize: attnT rows = outT * Zinv ; Z sits in psum row 64
            zis = {}
            for pp in range(2):
                for half in range(2):
                    zi = work.tile([65, 512], BF16, tag="rc", bufs=4)
                    _act_recip(nc, zi[64:65, :], pvt[(pp, half)][64:65, :])
                    zis[(pp, half)] = zi
            bcs_t = {}
            for pp in range(2):
                for half in range(2):
                    bc = psA.tile([HD, 512], F32, tag="sc", bufs=2)
                    nc.tensor.matmul(bc[:], ones64[64:65, :], zis[(pp, half)][64:65, :])
                    bcs = work.tile([HD, 512], F32, tag="bc", bufs=4)
                    nc.vector.tensor_copy(bcs[:], bc[:])
                    bcs_t[(pp, half)] = bcs
            for pp in range(2):
                p = pg * 2 + pp
                for half in range(2):
                    pv = pvt[(pp, half)]
                    bcs = bcs_t[(pp, half)]
                    if half == 0:
                        nc.vector.tensor_mul(attnT[0:HD, p * 512:(p + 1) * 512],
                                             pv[0:HD, :], bcs[:])
                    else:
                        tmpb = work.tile([HD, 512], BF16, tag="tmpb", bufs=4)
                        nc.vector.tensor_mul(tmpb[:], pv[0:HD, :], bcs[:])
                        # partition shift 0:64 -> 64:128 via sbuf-sbuf DMA
                        nc.sync.dma_start(attnT[HD:P, p * 512:(p + 1) * 512], tmpb[:])

        while fin_unit < 16:         # chunk0 has no pending final
            if attnT_prev is None:
                break
            emit_final_unit(c - 1, attnT_prev, fin_unit)
            fin_unit += 1
        if c + 1 < NSC:
            qT_cur = qTp.tile([P, NPAIR * 512], BF16, tag="qT", name=f"qT_{c+1}")
            emit_proj(c + 1, qT_cur)
        attnT_prev = attnT
    for unit in range(16):           # last chunk's output projection
        emit_final_unit(NSC - 1, attnT_prev, unit)


_NC_CACHE = {}


def build(S=2048):
    if S in _NC_CACHE:
        return _NC_CACHE[S]
    from contextlib import ExitStack
    nc = bacc.Bacc("TRN2", target_bir_lowering=False, debug=False, num_devices=8)
    with tile.TileContext(nc) as tc, ExitStack() as ctx:
        emit_kernel(nc, tc, ctx, S)
    nc.compile()
    _NC_CACHE[S] = nc
    return nc


def shard_inputs(x, theta, wq, wk, wv, wo, S=2048):
    """Returns in_maps for 8 cores: core = b*4 + g. Pure layout prep."""
    bf = ml_dtypes.bfloat16
    NSC = S // 512
    cost = np.cos(theta[:S]).astype(np.float32)
    sint = np.sin(theta[:S]).astype(np.float32)
    # [S, 32] -> [p, g*32+j]
    cosr = np.ascontiguousarray(cost.reshape(S // P, P, NJ).transpose(1, 0, 2)
                                ).reshape(P, -1)
    sinr = np.ascontiguousarray(sint.reshape(S // P, P, NJ).transpose(1, 0, 2)
                                ).reshape(P, -1)
    in_maps = []
    for core in range(8):
        b, g = core // 4, core % 4
        wq_g = wq[g * 512:(g + 1) * 512].reshape(8, HD, D)[HEAD_PERM].reshape(512, D)
        wo_g = wo[:, g * 512:(g + 1) * 512].reshape(D, 8, HD)[:, HEAD_PERM].reshape(D, 512)
        wqkv_g = np.concatenate([wq_g, wk[g * 128:(g + 1) * 128],
                                 wv[g * 128:(g + 1) * 128]], axis=0)   # [768, D]
        xT = np.ascontiguousarray(x[b, :S].T)                   # [d, s]
        # [d, s] -> [c*128+p, dt*512+sl]  (d = dt*128+p, s = c*512+sl)
        xT4 = np.ascontiguousarray(
            xT.reshape(DT, P, NSC, 512).transpose(2, 1, 0, 3)).reshape(NSC * P, -1)
        # w^T [d, o] -> [p, dt*ow+o]
        wqkvT = np.ascontiguousarray(
            wqkv_g.T.reshape(DT, P, OW).transpose(1, 0, 2)).reshape(P, -1)
        woT = np.ascontiguousarray(
            wo_g.T.reshape(NPAIR, P, D).transpose(1, 0, 2)).reshape(P, -1)
        in_maps.append({
            "xTv": xT4.astype(bf),
            "wqkv": wqkvT.astype(bf),
            "wo": woT.astype(bf),
            "cost": cosr,
            "sint": sinr,
        })
    return in_maps


def run_on_hw(inputs, S=2048, trace=False):
    nc = build(S)
    in_maps = shard_inputs(inputs["x"], inputs["theta"], inputs["wq"],
                           inputs["wk"], inputs["wv"], inputs["wo"], S=S)
    res = bass_utils.run_bass_kernel_spmd(nc, in_maps, core_ids=list(range(8)),
                                          trace=trace)
    parts = [res.results[c]["out"] for c in range(8)]
    out = np.stack([parts[0] + parts[1] + parts[2] + parts[3],
                    parts[4] + parts[5] + parts[6] + parts[7]], axis=0)
    return out, res


def kernel(x, theta, mask, wq, wk, wv, wo):
    out, _ = run_on_hw({"x": np.asarray(x, np.float32), "theta": np.asarray(theta, np.float32),
                        "wq": np.asarray(wq, np.float32), "wk": np.asarray(wk, np.float32),
                        "wv": np.asarray(wv, np.float32), "wo": np.asarray(wo, np.float32)})
    return out


# revision 17
# speedup vs baseline: 1.2040x; 1.2040x over previous
"""Trainium2 Bass kernel for GQA attention (B=2, S=2048, D=2048, H=32, KVH=8).

Sharding: 8 cores = 2 batches x 4 head-groups. Each core handles one batch and
8 q-heads / 2 kv-heads: wq/wk/wv column-parallel, wo row-parallel; the partial
wo products are summed on the host.

All matmuls bf16 (fp8 was tried: its quantization noise lands at ~1.8e-2 final
error vs the 2e-2 gate - no margin). The speed comes from keeping the PE
busy continuously (its clock ramps 1.2->2.4GHz only after ~3us without a gap):

  - Host packs every input as [128 x contiguous] blocks so each DMA is ~128
    big descriptors; chunk-0 data is queued first so compute starts early.
  - q/k/v projections are fused into one rhs (wqkv, N=768 per d-tile).
  - The causal diagonal is handled by accumulating a constant -10000
    triangular tile into the scores PSUM via a K=128 identity matmul, so
    exp -> pv has no cross-engine mask hop.
  - The attention inner loop is software-pipelined (scores(j) emitted before
    pv(j-1)) and one unit of the PREVIOUS chunk's output projection is
    interleaved after each j so the PE never drains while ACT runs exp.
  - softmax denominators: PV lhsT carries a ones column, so Z accumulates in
    PSUM row 64; recip on ACT (bf16), broadcast across partitions with a
    K=1 bf16 matmul.
"""

import os
import sys

for _p in ("/opt/trn_rl_repo", "/root/.axon_site/_ro/trn_rl_repo"):
    if os.path.isdir(_p) and _p not in sys.path:
        sys.path.append(_p)

import numpy as np
import ml_dtypes

import concourse.bass as bass
import concourse.mybir as mybir
import concourse.tile as tile
from concourse import bacc, bass_utils
from concourse.masks import make_identity

F32 = mybir.dt.float32
BF16 = mybir.dt.bfloat16
AFT = mybir.ActivationFunctionType

P = 128
D = 2048
HD = 64
NJ = HD // 2          # 32 rope freqs
OQ = 512              # q-head dims per core (8 heads * 64)
OKV = 128             # kv-head dims per core (2 heads * 64)
NPAIR = 4             # head pairs per core
DT = D // P           # 16 d-tiles
OW = OQ + 256         # fused qkv projection width (512 q + 128 k + 128 v)

HEAD_PERM = [0, 4, 1, 5, 2, 6, 3, 7]


def _act_recip(nc, out_ap, in_ap):
    eng = nc.scalar
    ins = [eng.lower_ap(in_ap),
           mybir.ImmediateValue(dtype=mybir.dt.float32, value=0.0),
           mybir.ImmediateValue(dtype=mybir.dt.float32, value=1.0),
           mybir.ImmediateValue(dtype=mybir.dt.float32, value=0.0)]
    return eng.add_instruction(mybir.InstActivation(
        name=nc.get_next_instruction_name(), func=AFT.Reciprocal,
        ins=ins, outs=[eng.lower_ap(out_ap)]))


def _emit_rope(nc, out_sb, in_ap, cos_ap, sin_ap, nh, tmp_pool, eng=None):
    """RoPE: out[.., 2j] = x0*c - x1*s ; out[.., 2j+1] = x0*s + x1*c.
    in_ap: [128, nh*64] bf16 (sbuf); out_sb: [128, nh*64] bf16;
    cos_ap/sin_ap: [128, 32] bf16 (per s-tile)."""
    if eng is None:
        eng = nc.vector
    w = nh * NJ
    x = in_ap.rearrange("p (h j t) -> p h j t", h=nh, j=NJ, t=2)
    o = out_sb.rearrange("p (h j t) -> p h j t", h=nh, j=NJ, t=2)
    x0, x1 = x[:, :, :, 0], x[:, :, :, 1]
    o0, o1 = o[:, :, :, 0], o[:, :, :, 1]
    c = cos_ap.unsqueeze(1).broadcast_to([P, nh, NJ])
    s = sin_ap.unsqueeze(1).broadcast_to([P, nh, NJ])
    tag = "rope_g" if eng is not nc.vector else "rope_v"
    ta = tmp_pool.tile([P, w], BF16, tag=tag + "a")
    tb = tmp_pool.tile([P, w], BF16, tag=tag + "b")
    ta3 = ta.rearrange("p (h j) -> p h j", h=nh, j=NJ)
    tb3 = tb.rearrange("p (h j) -> p h j", h=nh, j=NJ)
    eng.tensor_mul(ta3, x0, c)
    eng.tensor_mul(tb3, x1, s)
    eng.tensor_sub(o0, ta3, tb3)
    eng.tensor_mul(ta3, x0, s)
    eng.tensor_mul(tb3, x1, c)
    eng.tensor_add(o1, ta3, tb3)


def emit_kernel(nc, tc, ctx, S):
    NSC = S // 512        # s-chunks
    NST = S // P          # s-tiles (global)
    CW = DT * 512         # free width of one chunk of xT in sbuf

    xTv_d = nc.dram_tensor("xTv", [NSC * P, CW], BF16, kind="ExternalInput").ap()
    wqkv_d = nc.dram_tensor("wqkv", [P, DT * OW], BF16, kind="ExternalInput").ap()
    wo_d = nc.dram_tensor("wo", [P, NPAIR * D], BF16, kind="ExternalInput").ap()
    cos_d = nc.dram_tensor("cost", [P, NST * NJ], BF16, kind="ExternalInput").ap()
    sin_d = nc.dram_tensor("sint", [P, NST * NJ], BF16, kind="ExternalInput").ap()
    out_d = nc.dram_tensor("out", [S, D], F32, kind="ExternalOutput").ap()

    ctx.enter_context(nc.allow_low_precision(reason="bf16 tiles feed matmuls"))
    const = ctx.enter_context(tc.tile_pool(name="const", bufs=1))
    work = ctx.enter_context(tc.tile_pool(name="work", bufs=2))
    epool = ctx.enter_context(tc.tile_pool(name="epool", bufs=6))
    qTp = ctx.enter_context(tc.tile_pool(name="qTp", bufs=2))
    atp = ctx.enter_context(tc.tile_pool(name="atp", bufs=2))
    psA = ctx.enter_context(tc.tile_pool(name="psA", bufs=2, space="PSUM"))
    psB = ctx.enter_context(tc.tile_pool(name="psB", bufs=4, space="PSUM"))

    idn = const.tile([P, P], BF16)
    make_identity(nc, idn)
    triU = const.tile([P, P], BF16)         # -10000 where k > q, else 0
    nc.gpsimd.memset(triU[:], -10000.0)
    nc.gpsimd.affine_select(out=triU[:], in_=triU[:],
                            compare_op=mybir.AluOpType.is_ge, fill=0.0,
                            base=-1, channel_multiplier=1, pattern=[[-1, P]])
    ones_f = const.tile([P, 1], F32)
    nc.any.memset(ones_f[:], 1.0)
    ones64 = const.tile([65, HD], BF16)
    nc.vector.tensor_copy(ones64[:], ones_f[0:65, 0:1].broadcast_to([65, HD]))

    xTv = const.tile([P, NSC * CW], BF16)   # [p, c*CW + dt*512 + sl]
    wqkv = const.tile([P, DT * OW], BF16)   # [p, dt*768 + (q512|k128|v128)]
    wo = const.tile([P, NPAIR * D], BF16)   # [p, pp*2048 + d]
    kT = const.tile([P, S], BF16)           # [o_kv, s]
    v2 = const.tile([P, NST * 130], BF16)   # [s_loc, g*130 + a*65 + (hd|one)]
    cosr = const.tile([P, NST * NJ], BF16)
    sinr = const.tile([P, NST * NJ], BF16)

    # priority-ordered bulk loads (each side is [128, contiguous])
    nc.sync.dma_start(wqkv[:], wqkv_d)
    nc.sync.dma_start(xTv[:, 0:CW], xTv_d[0:P, :])
    nc.sync.dma_start(cosr[:], cos_d)
    nc.sync.dma_start(sinr[:], sin_d)
    for c in range(1, NSC):
        nc.sync.dma_start(xTv[:, c * CW:(c + 1) * CW], xTv_d[c * P:(c + 1) * P, :])
    nc.sync.dma_start(wo[:], wo_d)

    # ones columns of v2 (positions i*65 + 64)
    v2ones = v2[:].rearrange("p (i c) -> p i c", i=2 * NST, c=65)[:, :, 64]
    nc.vector.tensor_copy(v2ones, ones_f[:, 0:1].broadcast_to([P, 2 * NST]))

    wqkvv = wqkv[:].rearrange("p (dt o) -> p dt o", dt=DT, o=OW)

    # ---- fused q/k/v projection for one s-chunk ----
    def emit_proj(c, qT):
        xvc = xTv[:, c * CW:(c + 1) * CW].rearrange("p (dt s) -> p dt s", dt=DT, s=512)
        pjs, tps = [], []
        for st in range(4):
            sl = slice(st * P, (st + 1) * P)
            pj = psA.tile([P, 1024], F32, tag="sc", bufs=2, name=f"pj_{c}_{st}")
            for dt in range(DT):
                nc.tensor.matmul(pj[:, 0:512], xvc[:, dt, sl], wqkvv[:, dt, 0:512],
                                 start=(dt == 0), stop=(dt == DT - 1),
                                 skip_group_check=True)
            for dt in range(DT):
                nc.tensor.matmul(pj[:, 512:OW], xvc[:, dt, sl], wqkvv[:, dt, 512:OW],
                                 start=(dt == 0), stop=(dt == DT - 1),
                                 skip_group_check=True)
            pjs.append(pj)
            if st > 0:
                emit_rope_tp(c, st - 1, pjs[st - 1], tps)
        emit_rope_tp(c, 3, pjs[3], tps)
        for st in range(4):
            emit_tp_copy(c, st, qT, tps[st])

    def emit_rope_tp(c, st, pj, tps):
        g = c * 4 + st
        cos_ap = cosr[:, g * NJ:(g + 1) * NJ]
        sin_ap = sinr[:, g * NJ:(g + 1) * NJ]
        # stage q|k to bf16 SBUF on ACT: DVE/gpsimd rope then runs at 16-bit
        # rate with cheap SBUF access
        qk = work.tile([P, 640], BF16, tag="qk")
        nc.scalar.copy(qk[:], pj[:, 0:640])
        qr = work.tile([P, OQ], BF16, tag="qr")
        _emit_rope(nc, qr[:], qk[:, 0:512], cos_ap, sin_ap, 8, work)
        kr = work.tile([P, OKV], BF16, tag="kr")
        _emit_rope(nc, kr[:], qk[:, 512:640], cos_ap, sin_ap, 2, work,
                   eng=nc.gpsimd)
        v_src = pj[:, 640:768].rearrange("p (a x) -> p a x", a=2, x=HD)
        v_dst = v2[:, g * 130:(g + 1) * 130].rearrange("p (a x) -> p a x",
                                                       a=2, x=65)[:, :, 0:HD]
        nc.vector.tensor_copy(v_dst, v_src)
        # transposes into a borrowed psB slot (bf16, 1 cycle/row)
        tp = psB.tile([P, 640], BF16, tag="pv", bufs=4, name=f"tp_{c}_{st}")
        for p in range(NPAIR):
            nc.tensor.transpose(tp[:, p * P:(p + 1) * P], qr[:, p * P:(p + 1) * P], idn[:])
        nc.tensor.transpose(tp[:, 512:640], kr[:], idn[:])
        tps.append(tp)

    def emit_tp_copy(c, st, qT, tp):
        g = c * 4 + st
        for p in range(NPAIR):
            nc.vector.tensor_copy(qT[:, p * 512 + st * P: p * 512 + (st + 1) * P],
                                  tp[:, p * P:(p + 1) * P])
        nc.vector.tensor_copy(kT[:, g * P:(g + 1) * P], tp[:, 512:640])

    def emit_pv(pvt, prev, NJT):
        j, vs, e2s = prev
        for pp in range(2):
            e2 = e2s[pp]
            nc.tensor.matmul(pvt[(pp, 0)][:, vs:512],
                             v2[:, j * 130: j * 130 + 65],
                             e2[:, vs:512],
                             start=(j == 0), stop=(j == NJT - 1), skip_group_check=True)
            nc.tensor.matmul(pvt[(pp, 1)][:, vs:512],
                             v2[:, j * 130 + 65: (j + 1) * 130],
                             e2[:, 512 + vs:1024],
                             start=(j == 0), stop=(j == NJT - 1), skip_group_check=True)

    def emit_final_unit(fc, attnT_f, unit):
        st, dc = divmod(unit, 4)
        rp = psA.tile([P, 1024], F32, tag="sc", bufs=2, name=f"rp_{fc}_{unit}")
        rpv = rp[:, 0:512]
        for p in range(NPAIR):
            nc.tensor.matmul(rpv, attnT_f[:, p * 512 + st * P: p * 512 + (st + 1) * P],
                             wo[:, p * D + dc * 512: p * D + (dc + 1) * 512],
                             start=(p == 0), stop=(p == NPAIR - 1),
                             skip_group_check=True)
        rs = work.tile([P, 512], F32, tag="rs")
        if unit % 2 == 0:
            nc.vector.tensor_copy(rs[:], rpv)
        else:
            nc.scalar.copy(rs[:], rpv)
        nc.sync.dma_start(out_d[(fc * 4 + st) * P:(fc * 4 + st + 1) * P,
                                dc * 512:(dc + 1) * 512], rs[:])

    # ---- main loop over s-chunks ----
    qT_cur = qTp.tile([P, NPAIR * 512], BF16, tag="qT", name="qT_0")
    emit_proj(0, qT_cur)
    attnT_prev = None
    for c in range(NSC):
        qT = qT_cur
        attnT = atp.tile([P, NPAIR * 512], BF16, tag="attnT")
        NJT = 4 * (c + 1)
        fin = {"u": 0 if attnT_prev is not None else 16}

        def fill_unit():
            if fin["u"] < 16:
                emit_final_unit(c - 1, attnT_prev, fin["u"])
                fin["u"] += 1

        for pg in range(2):          # two pair-groups, 2 head-pairs each
            pvt = {}
            for pp in range(2):
                for half in range(2):
                    pvt[(pp, half)] = psB.tile([65, 512], F32, tag="pv", bufs=4,
                                               name=f"pv_{c}_{pg}_{pp}_{half}")
            prev = None              # (j, vs, {pp: e2})
            for j in range(NJT):
                vs = max(0, (j - 4 * c) * P)
                e2s = {}
                for pp in range(2):
                    p = pg * 2 + pp
                    sc2 = psA.tile([P, 1024], F32, tag="sc", bufs=2)
                    nc.tensor.matmul(sc2[:, vs:512], kT[0:HD, j * P:(j + 1) * P],
                                     qT[0:HD, p * 512 + vs:(p + 1) * 512])
                    nc.tensor.matmul(sc2[:, 512 + vs:1024], kT[HD:P, j * P:(j + 1) * P],
                                     qT[HD:P, p * 512 + vs:(p + 1) * 512])
                    if j >= 4 * c:   # diagonal: accumulate -10000 upper-tri
                        for half in range(2):
                            nc.tensor.matmul(sc2[:, half * 512 + vs: half * 512 + vs + P],
                                             idn[:], triU[:],
                                             start=False, stop=True,
                                             skip_group_check=True)
                    e2 = epool.tile([P, 1024], BF16, tag="e", bufs=6)
                    if vs:
                        sc_v = sc2[:].rearrange("p (h q) -> p h q", h=2, q=512)[:, :, vs:512]
                        e_v = e2[:].rearrange("p (h q) -> p h q", h=2, q=512)[:, :, vs:512]
                        nc.scalar.activation(e_v, sc_v, AFT.Exp, scale=1.0 / 8.0)
                    else:
                        nc.scalar.activation(e2[:], sc2[:], AFT.Exp, scale=1.0 / 8.0)
                    e2s[pp] = e2
                if prev is not None:
                    emit_pv(pvt, prev, NJT)
                prev = (j, vs, e2s)
                if fin["u"] < 12:    # keep 4 units back as normalize fillers
                    fill_unit()
            emit_pv(pvt, prev, NJT)
            # normalize: attnT rows = outT * Zinv ; Z sits in psum row 64
            zis = {}
            for pp in range(2):
                for half in range(2):
                    zi = work.tile([65, 512], BF16, tag="rc", bufs=4)
                    _act_recip(nc, zi[64:65, :], pvt[(pp, half)][64:65, :])
                    zis[(pp, half)] = zi
            fill_unit()              # PE work while ACT runs the recips
            bcs_t = {}
            for pp in range(2):
                for half in range(2):
                    bc = psA.tile([HD, 512], F32, tag="sc", bufs=2)
                    nc.tensor.matmul(bc[:], ones64[64:65, :], zis[(pp, half)][64:65, :])
                    bcs = work.tile([HD, 512], F32, tag="bc", bufs=4)
                    nc.vector.tensor_copy(bcs[:], bc[:])
                    bcs_t[(pp, half)] = bcs
            fill_unit()
            for pp in range(2):
                p = pg * 2 + pp
                for half in range(2):
                    pv = pvt[(pp, half)]
                    bcs = bcs_t[(pp, half)]
                    if half == 0:
                        nc.vector.tensor_mul(attnT[0:HD, p * 512:(p + 1) * 512],
                                             pv[0:HD, :], bcs[:])
                    else:
                        tmpb = work.tile([HD, 512], BF16, tag="tmpb", bufs=4)
                        nc.vector.tensor_mul(tmpb[:], pv[0:HD, :], bcs[:])
                        # partition shift 0:64 -> 64:128 via sbuf-sbuf DMA
                        nc.sync.dma_start(attnT[HD:P, p * 512:(p + 1) * 512], tmpb[:])

        while fin["u"] < 16:         # leftovers (chunk0 has none pending)
            fill_unit()
        if c + 1 < NSC:
            qT_cur = qTp.tile([P, NPAIR * 512], BF16, tag="qT", name=f"qT_{c+1}")
            emit_proj(c + 1, qT_cur)
        attnT_prev = attnT
    for unit in range(16):           # last chunk's output projection
        emit_final_unit(NSC - 1, attnT_prev, unit)


_NC_CACHE = {}


def build(S=2048):
    if S in _NC_CACHE:
        return _NC_CACHE[S]
    from contextlib import ExitStack
    nc = bacc.Bacc("TRN2", target_bir_lowering=False, debug=False, num_devices=8)
    with tile.TileContext(nc) as tc, ExitStack() as ctx:
        emit_kernel(nc, tc, ctx, S)
    nc.compile()
    _NC_CACHE[S] = nc
    return nc


def shard_inputs(x, theta, wq, wk, wv, wo, S=2048):
    """Returns in_maps for 8 cores: core = b*4 + g. Pure layout prep."""
    bf = ml_dtypes.bfloat16
    NSC = S // 512
    cost = np.cos(theta[:S]).astype(np.float32)
    sint = np.sin(theta[:S]).astype(np.float32)
    # [S, 32] -> [p, g*32+j]
    cosr = np.ascontiguousarray(cost.reshape(S // P, P, NJ).transpose(1, 0, 2)
                                ).reshape(P, -1)
    sinr = np.ascontiguousarray(sint.reshape(S // P, P, NJ).transpose(1, 0, 2)
                                ).reshape(P, -1)
    in_maps = []
    for core in range(8):
        b, g = core // 4, core % 4
        wq_g = wq[g * 512:(g + 1) * 512].reshape(8, HD, D)[HEAD_PERM].reshape(512, D)
        wo_g = wo[:, g * 512:(g + 1) * 512].reshape(D, 8, HD)[:, HEAD_PERM].reshape(D, 512)
        wqkv_g = np.concatenate([wq_g, wk[g * 128:(g + 1) * 128],
                                 wv[g * 128:(g + 1) * 128]], axis=0)   # [768, D]
        xT = np.ascontiguousarray(x[b, :S].T)                   # [d, s]
        # [d, s] -> [c*128+p, dt*512+sl]  (d = dt*128+p, s = c*512+sl)
        xT4 = np.ascontiguousarray(
            xT.reshape(DT, P, NSC, 512).transpose(2, 1, 0, 3)).reshape(NSC * P, -1)
        # w^T [d, o] -> [p, dt*ow+o]
        wqkvT = np.ascontiguousarray(
            wqkv_g.T.reshape(DT, P, OW).transpose(1, 0, 2)).reshape(P, -1)
        woT = np.ascontiguousarray(
            wo_g.T.reshape(NPAIR, P, D).transpose(1, 0, 2)).reshape(P, -1)
        in_maps.append({
            "xTv": xT4.astype(bf),
            "wqkv": wqkvT.astype(bf),
            "wo": woT.astype(bf),
            "cost": cosr.astype(bf),
            "sint": sinr.astype(bf),
        })
    return in_maps


def run_on_hw(inputs, S=2048, trace=False):
    nc = build(S)
    in_maps = shard_inputs(inputs["x"], inputs["theta"], inputs["wq"],
                           inputs["wk"], inputs["wv"], inputs["wo"], S=S)
    res = bass_utils.run_bass_kernel_spmd(nc, in_maps, core_ids=list(range(8)),
                                          trace=trace)
    parts = [res.results[c]["out"] for c in range(8)]
    out = np.stack([parts[0] + parts[1] + parts[2] + parts[3],
                    parts[4] + parts[5] + parts[6] + parts[7]], axis=0)
    return out, res


def kernel(x, theta, mask, wq, wk, wv, wo):
    out, _ = run_on_hw({"x": np.asarray(x, np.float32), "theta": np.asarray(theta, np.float32),
                        "wq": np.asarray(wq, np.float32), "wk": np.asarray(wk, np.float32),
                        "wv": np.asarray(wv, np.float32), "wo": np.asarray(wo, np.float32)})
    return out
